# revision 9
# baseline (speedup 1.0000x reference)
"""Trainium2 Bass kernel for nn_CapsuleLayer_9852654977072.

The reference module collapses mathematically: the routing loop's coupling
logits `b` stay zero (faithfully-reproduced bug in the original torch code),
so routing coefficients are a fixed spatial map r(h,w) = 1/(8*cnt(h,w)) where
cnt is the 5x5 box-count inside the image. The whole module is therefore:

    p = conv2d(u as [N,64,H,W], Wd as [128,64,5,5], pad=2) * s(h,w)
    v = squash_z1(p)   # groups of 16 channels
    out[n,t1,z1,h,w] = v

Device strategy (8 cores, SPMD): shard (batch n in 0..3) x (row-half in 0..1).
Each core computes all 128 output channels for 64 rows of one image.

Conv: inputs shipped as XA/XC (partition halves hold u shifted by (+0,+1) rows
and (+2row,+0/+1col) respectively, columns padded by 2), chunked into
per-2-block SBUF tiles so the first conv matmul only waits for one small DMA
(tile deps are whole-tile). Per 4-row block, 13 PSUM-accumulated fp32r matmuls
(N=512, full PE rate) cover all 25 taps.

Squash: per group of blocks, block-diagonal matmuls pack m2 = sum_z1 q^2 for
all (block, t1) pairs into one [8*ng, 512] PSUM tile. The factor
F = y/((1+y)*sqrt(y_raw+eps)), y = s^2*y_raw runs ONCE per group on ACT/DVE
(no GpSimd), then expand matmuls broadcast F back to the 128 channels and
v = p * F. Expansion of group g-1 is emitted after the conv of group g so the
PE queue never stalls on the factor chain; groups are [4,4,4,2,2] so the tail
factor latency hides behind a 2-block conv. Dummy matmuls at kernel start
keep the PE busy during the DMA lead-in (HAM stays un-throttled), and DMA
issue is spread across the scalar/vector/sync queues (descriptor generation
is ~850ns per dma_start on the issuing queue).
"""

import numpy as np

T0, Z0, T1, Z1, KK, PAD = 4, 16, 8, 16, 5, 2
N, H, W_SP = 4, 128, 128
CIN, COUT = T0 * Z0, T1 * Z1  # 64, 128
N_CORES = 8
ROWS = 64          # output rows per core
XROWS = 68         # input rows incl. halo
XCOLS = 132        # 128 + 2*PAD
BLK = 4            # output rows per block
N_BLKS = ROWS // BLK   # 16
GROUPS = [(0, 4), (4, 4), (8, 4), (12, 2), (14, 2)]  # (first block, n blocks)

# conv matmul j -> (source, row_off, col_off); weights match in _weight_tiles
_MM_SLICES = (
    [('XA', dy + 2, dx + 2) for dy in (-2, 0) for dx in (-2, -1, 0, 1, 2)]
    + [('XC', 2, 0), ('XC', 2, 2), ('XC', 2, 4)]
)

_CACHE = {}


def _bp_of(blk):
    for b0, ng in GROUPS:
        if b0 <= blk < b0 + ng:
            return blk - b0
    raise ValueError(blk)


def _weight_tiles(W):
    Wd = W.transpose(1, 0, 2, 3, 4).reshape(COUT, CIN, KK, KK)
    wl = np.zeros((128, 13, 128), np.float32)  # [k, j, m]
    j = 0
    for dy in (-2, 0):
        for dx in (-2, -1, 0, 1, 2):
            wl[0:64, j, :] = Wd[:, :, dy + 2, dx + 2].T
            wl[64:128, j, :] = Wd[:, :, dy + 3, dx + 2].T
            j += 1
    for dx0 in (-2, 0):
        wl[0:64, j, :] = Wd[:, :, 4, dx0 + 2].T
        wl[64:128, j, :] = Wd[:, :, 4, dx0 + 3].T
        j += 1
    wl[0:64, j, :] = Wd[:, :, 4, 4].T  # single tap (2,2) on lo partitions
    return wl


def _inputs_core(x, half):
    """x: [64, H, W] one image channel-major. Returns XA, XC [128, 68, 132]."""
    base = half * 64 - 2
    XA = np.zeros((128, XROWS, XCOLS), np.float32)
    XC = np.zeros((128, XROWS, XCOLS), np.float32)

    def fill(dst, roff, c0, c1):
        lo, hi = max(0, -(base + roff)), min(XROWS, H - base - roff)
        dst[:, lo:hi, c0:c1] = x[:, base + roff + lo:base + roff + hi, :]

    fill(XA[0:64], 0, 2, 130)
    fill(XA[64:128], 1, 2, 130)
    fill(XC[0:64], 2, 2, 130)
    fill(XC[64:128], 2, 1, 129)
    return XA, XC


def _s2_groups(half):
    """[32, len(GROUPS), BLK*128] f32: s^2 at partition m=8*bp+t1
    (t1-replicated), group g, flat pos = (row-within-block, col)."""
    idx = np.arange(H)
    cnt = (np.minimum(idx + 2, H - 1) - np.maximum(idx - 2, 0) + 1).astype(np.float64)
    s = 1.0 / (8.0 * cnt[:, None] * cnt[None, :])  # [H, W]
    s2 = (s * s)[half * 64:(half + 1) * 64, :]     # [64, 128]
    out = np.zeros((32, len(GROUPS), BLK * 128), np.float64)
    for g, (b0, ng) in enumerate(GROUPS):
        for bp in range(ng):
            blk = b0 + bp
            rows = s2[blk * BLK:(blk + 1) * BLK, :].reshape(-1)  # [512]
            out[8 * bp:8 * bp + 8, g, :] = rows[None, :]
    return np.ascontiguousarray(
        out.astype(np.float32).reshape(32, len(GROUPS) * BLK * 128))


def _bdv():
    """[128, N_BLKS*32]: c=(t1,z1) -> partition m=8*bp(blk)+t1, sum over z1."""
    bd = np.zeros((128, N_BLKS, 32), np.float32)
    c = np.arange(128)
    for blk in range(N_BLKS):
        bd[c, blk, 8 * _bp_of(blk) + c // 16] = 1.0
    return np.ascontiguousarray(bd.reshape(128, N_BLKS * 32))


def _exv():
    """[32, N_BLKS*128]: partition p=8*bp(blk)+t1 -> channels c, c//16==t1."""
    ex = np.zeros((32, N_BLKS, 128), np.float32)
    c = np.arange(128)
    for blk in range(N_BLKS):
        ex[8 * _bp_of(blk) + c // 16, blk, c] = 1.0
    return np.ascontiguousarray(ex.reshape(32, N_BLKS * 128))


def build_nc(reps=1):
    import concourse.bass as bass
    import concourse.bacc as bacc
    import concourse.mybir as mybir
    import concourse.tile as tile

    f32 = mybir.dt.float32
    f32r = mybir.dt.float32r
    AF = mybir.ActivationFunctionType
    NG = len(GROUPS)

    nc = bacc.Bacc(None, target_bir_lowering=False)
    xa_d = nc.dram_tensor("xa", [128, XROWS * XCOLS], f32r, kind="ExternalInput")
    xc_d = nc.dram_tensor("xc", [128, XROWS * XCOLS], f32r, kind="ExternalInput")
    wl_d = nc.dram_tensor("wl", [128, 13 * 128], f32r, kind="ExternalInput")
    bdv_d = nc.dram_tensor("bdv", [128, N_BLKS * 32], f32r, kind="ExternalInput")
    exv_d = nc.dram_tensor("exv", [32, N_BLKS * 128], f32r, kind="ExternalInput")
    s2_d = nc.dram_tensor("s2", [32, NG * BLK * 128], f32, kind="ExternalInput")
    out_d = nc.dram_tensor("out", [128, ROWS * 128], f32, kind="ExternalOutput")

    with tile.TileContext(nc) as tc:
        with (
            tc.tile_pool(name="consts", bufs=1) as consts,
            tc.tile_pool(name="sq", bufs=3) as sq,
            tc.tile_pool(name="psb", bufs=9) as psb,
            tc.tile_pool(name="fac", bufs=2) as fac,
            tc.tile_pool(name="ff", bufs=2) as ff,
            tc.tile_pool(name="vv", bufs=3) as vv,
            tc.tile_pool(name="pp", bufs=3, space="PSUM") as pp,
            tc.tile_pool(name="py", bufs=2, space="PSUM") as py,
            tc.tile_pool(name="pf", bufs=2, space="PSUM") as pf,
            tc.tile_pool(name="pd", bufs=1, space="PSUM") as pd,
        ):
            # PE pre-warm: dummy matmuls on a zeroed tile keep the PE busy
            # during the input-DMA lead-in so HAM un-throttles to 2.4 GHz
            # before the first real conv matmul.
            dum = consts.tile([128, 512], f32)
            nc.gpsimd.memset(dum[:], 0.0)
            dum_ps = pd.tile([128, 512], f32)
            for _ in range(14):
                nc.tensor.matmul(dum_ps[:], dum[:, 0:128].bitcast(f32r),
                                 dum[:].bitcast(f32r), start=True, stop=True)

            wl = consts.tile([128, 13, 128], f32r)
            wl_src = wl_d.ap().rearrange("p (j m) -> p j m", m=128)
            nc.sync.dma_start(out=wl, in_=wl_src)

            # Input chunks: one tile per 2 blocks so the conv can start as
            # soon as the first small chunk lands. xa chunk i holds source
            # rows [8i, 8i+10); xc chunk i holds rows [8i+2, 8i+10).
            xa_src = xa_d.ap().rearrange("p (r c) -> p r c", c=XCOLS)
            xc_src = xc_d.ap().rearrange("p (r c) -> p r c", c=XCOLS)
            xat = [consts.tile([128, 10, XCOLS], f32r, name=f"xat{i}")
                   for i in range(8)]
            xct = [consts.tile([128, 8, XCOLS], f32r, name=f"xct{i}")
                   for i in range(8)]
            nc.sync.dma_start(out=xat[0][:], in_=xa_src[:, 0:10, :])
            nc.sync.dma_start(out=xct[0][:], in_=xc_src[:, 2:10, :])
            nc.sync.dma_start(out=xat[1][:], in_=xa_src[:, 8:18, :])
            nc.sync.dma_start(out=xct[1][:], in_=xc_src[:, 10:18, :])
            # Priority gate: hold the bulk-chunk DMA issue (scalar queue)
            # until the critical first-block bytes have landed, so they get
            # full HBM bandwidth.
            gate_t = consts.tile([1, 4], f32)
            nc.scalar.activation(gate_t[:, 0:1], wl[0:1, 0, 0:1].bitcast(f32),
                                 AF.Copy, bias=0.0)
            nc.scalar.activation(gate_t[:, 1:2],
                                 xat[0][0:1, 0, 0:1].bitcast(f32),
                                 AF.Copy, bias=0.0)
            nc.scalar.activation(gate_t[:, 2:3],
                                 xct[0][0:1, 0, 0:1].bitcast(f32),
                                 AF.Copy, bias=0.0)
            for i in range(2, 8):
                nc.scalar.dma_start(
                    out=xat[i][:], in_=xa_src[:, 8 * i:8 * i + 10, :])
                nc.scalar.dma_start(
                    out=xct[i][:], in_=xc_src[:, 8 * i + 2:8 * i + 10, :])

            bdv = consts.tile([128, N_BLKS, 32], f32r)
            nc.sync.dma_start(
                out=bdv, in_=bdv_d.ap().rearrange("p (b m) -> p b m", m=32))
            exv = consts.tile([32, N_BLKS, 128], f32r)
            nc.sync.dma_start(
                out=exv, in_=exv_d.ap().rearrange("p (b c) -> p b c", c=128))
            s2_sb = consts.tile([32, NG, BLK, 128], f32)
            nc.sync.dma_start(
                out=s2_sb,
                in_=s2_d.ap().rearrange("p (g r c) -> p g r c", r=BLK, c=128))
            eps_t = consts.tile([32, 1], f32)
            nc.gpsimd.memset(eps_t[:], 1e-9)

            out_v = out_d.ap().rearrange("p (r c) -> p r c", c=128)

            import contextlib
            loop_ctx = (tc.For_i(0, reps, 1,
                                 hint_engines=(mybir.EngineType.PE,
                                               mybir.EngineType.DVE,
                                               mybir.EngineType.Activation,
                                               mybir.EngineType.Pool,
                                               mybir.EngineType.SP))
                        if reps > 1 else contextlib.nullcontext())

            def conv_group(gi, b0, ng, psbs):
                y_ps = py.tile([8 * ng, BLK, 128], f32)
                for bp in range(ng):
                    blk = b0 + bp
                    ci, ro = blk // 2, 4 * (blk % 2)
                    p_ps = pp.tile([128, BLK, 128], f32)
                    for j, (src, roff, coff) in enumerate(_MM_SLICES):
                        if src == 'XA':
                            xt, r = xat[ci], ro + roff
                        else:
                            xt, r = xct[ci], ro + roff - 2
                        if j == 12:  # K=64 single on lo partitions
                            lhsT = wl[0:64, j, :]
                            rhs = xt[0:64, r:r + BLK, coff:coff + 128]
                        else:
                            lhsT = wl[:, j, :]
                            rhs = xt[:, r:r + BLK, coff:coff + 128]
                        nc.tensor.matmul(p_ps[:], lhsT, rhs,
                                         start=(j == 0), stop=(j == 12))
                    psq = sq.tile([128, BLK, 128], f32r, tag="psq")
                    nc.scalar.activation(psq[:], p_ps[:], AF.Square)
                    p_sb = psb.tile([128, BLK, 128], f32, tag="psb")
                    nc.scalar.activation(p_sb[:], p_ps[:], AF.Copy, bias=0.0)
                    psbs[blk] = p_sb
                    nc.tensor.matmul(y_ps[:], bdv[:, blk, 0:8 * ng], psq[:],
                                     start=(bp == 0), stop=(bp == ng - 1))
                return y_ps

            def factor(gi, ng, y_ps):
                # F = y/((1+y)*sqrt(y_raw+eps)), y = s^2*y_raw, on [8ng, 512]
                P = 8 * ng
                a_t = fac.tile([P, BLK, 128], f32, tag="a")
                nc.scalar.activation(a_t[:], y_ps[:], AF.Sqrt,
                                     bias=eps_t[0:P, :])
                y_t = fac.tile([P, BLK, 128], f32, tag="y")
                nc.vector.tensor_mul(y_t[:], y_ps[:], s2_sb[0:P, gi, :, :])
                y1_t = fac.tile([P, BLK, 128], f32, tag="y1")
                nc.scalar.activation(y1_t[:], y_t[:], AF.Copy, bias=1.0)
                b_t = fac.tile([P, BLK, 128], f32, tag="b")
                nc.vector.tensor_mul(b_t[:], a_t[:], y1_t[:])
                r_t = fac.tile([P, BLK, 128], f32, tag="r")
                nc.vector.reciprocal_approx_fast(r_t[:], b_t[:])
                F_t = ff.tile([P, BLK, 128], f32r, tag="F")
                nc.vector.tensor_mul(F_t[:], y_t[:], r_t[:])
                return F_t

            def expand(b0, ng, F_t, psbs):
                for bp in range(ng):
                    blk = b0 + bp
                    r0 = blk * BLK
                    fe_ps = pf.tile([128, BLK, 128], f32)
                    nc.tensor.matmul(fe_ps[:], exv[0:8 * ng, blk, :], F_t[:],
                                     start=True, stop=True)
                    v_t = vv.tile([128, BLK, 128], f32, tag="v")
                    nc.vector.tensor_mul(v_t[:], psbs.pop(blk)[:], fe_ps[:])
                    nc.sync.dma_start(out=out_v[:, r0:r0 + BLK, :], in_=v_t[:])

            with loop_ctx:
                psbs = {}
                prev = None
                for gi, (b0, ng) in enumerate(GROUPS):
                    y_ps = conv_group(gi, b0, ng, psbs)
                    if prev is not None:
                        expand(prev[0], prev[1], prev[2], psbs)
                    prev = (b0, ng, factor(gi, ng, y_ps))
                expand(prev[0], prev[1], prev[2], psbs)

    nc.compile()
    return nc


def _prep_in_maps(u, W):
    x = u.reshape(N, CIN, H, W_SP)
    wl = _weight_tiles(W).reshape(128, 13 * 128)
    bdv = _bdv()
    exv = _exv()
    s2q = [_s2_groups(half) for half in range(2)]
    in_maps = []
    for core in range(N_CORES):
        n, half = core // 2, core % 2
        XA, XC = _inputs_core(x[n], half)
        in_maps.append({
            "xa": XA.reshape(128, XROWS * XCOLS),
            "xc": XC.reshape(128, XROWS * XCOLS),
            "wl": wl,
            "bdv": bdv,
            "exv": exv,
            "s2": s2q[half],
        })
    return in_maps


def run(u, W, trace=False):
    """Returns (out [N,T1,Z1,H,W] f32, BassKernelResults)."""
    from concourse.bass_utils import run_bass_kernel_spmd

    if "nc" not in _CACHE:
        _CACHE["nc"] = build_nc()
    nc = _CACHE["nc"]
    in_maps = _prep_in_maps(np.asarray(u, np.float32), np.asarray(W, np.float32))
    res = run_bass_kernel_spmd(nc, in_maps, list(range(N_CORES)), trace=trace)
    out = np.empty((N, T1, Z1, H, W_SP), np.float32)
    for core in range(N_CORES):
        n, half = core // 2, core % 2
        o = res.results[core]["out"].reshape(T1, Z1, ROWS, 128)
        out[n, :, :, half * 64:(half + 1) * 64, :] = o
    return out, res


def kernel(u, W):
    out, _ = run(u, W, trace=False)
    return out


# revision 11
# speedup vs baseline: 1.0732x; 1.0732x over previous
"""Trainium2 Bass kernel for nn_CapsuleLayer_9852654977072.

The reference module collapses mathematically: the routing loop's coupling
logits `b` stay zero (faithfully-reproduced bug in the original torch code),
so routing coefficients are a fixed spatial map r(h,w) = 1/(8*cnt(h,w)) where
cnt is the 5x5 box-count inside the image. The whole module is therefore:

    p = conv2d(u as [N,64,H,W], Wd as [128,64,5,5], pad=2) * s(h,w)
    v = squash_z1(p)   # groups of 16 channels
    out[n,t1,z1,h,w] = v

Device strategy (8 cores, SPMD): shard (batch n in 0..3) x (row-half in 0..1).
Each core computes all 128 output channels for 64 rows of one image.

Conv: inputs shipped as XA/XC (partition halves hold u shifted by (+0,+1) rows
and (+2row,+0/+1col) respectively, columns padded by 2), chunked into
per-2-block SBUF tiles so the first conv matmul only waits for one small DMA
(tile deps are whole-tile). Per 4-row block, 13 PSUM-accumulated fp32r matmuls
(N=512, full PE rate) cover all 25 taps.

Squash: per group of blocks, block-diagonal matmuls pack m2 = sum_z1 q^2 for
all (block, t1) pairs into one [8*ng, 512] PSUM tile. The factor
F = y/((1+y)*sqrt(y_raw+eps)), y = s^2*y_raw runs ONCE per group on ACT/DVE
(no GpSimd), then expand matmuls broadcast F back to the 128 channels and
v = p * F. Expansion of group g-1 is emitted after the conv of group g so the
PE queue never stalls on the factor chain; groups are [4,4,4,2,2] so the tail
factor latency hides behind a 2-block conv. Dummy matmuls at kernel start
keep the PE busy during the DMA lead-in (HAM stays un-throttled), and DMA
issue is spread across the scalar/vector/sync queues (descriptor generation
is ~850ns per dma_start on the issuing queue).
"""

import numpy as np

T0, Z0, T1, Z1, KK, PAD = 4, 16, 8, 16, 5, 2
N, H, W_SP = 4, 128, 128
CIN, COUT = T0 * Z0, T1 * Z1  # 64, 128
N_CORES = 8
ROWS = 64          # output rows per core
XROWS = 68         # input rows incl. halo
XCOLS = 132        # 128 + 2*PAD
BLK = 4            # output rows per block
N_BLKS = ROWS // BLK   # 16
GROUPS = [(0, 4), (4, 4), (8, 4), (12, 2), (14, 2)]  # (first block, n blocks)

# conv matmul j -> (source, row_off, col_off); weights match in _weight_tiles
_MM_SLICES = (
    [('XA', dy + 2, dx + 2) for dy in (-2, 0) for dx in (-2, -1, 0, 1, 2)]
    + [('XC', 2, 0), ('XC', 2, 2), ('XC', 2, 4)]
)

_CACHE = {}


def _bp_of(blk):
    for b0, ng in GROUPS:
        if b0 <= blk < b0 + ng:
            return blk - b0
    raise ValueError(blk)


def _weight_tiles(W):
    Wd = W.transpose(1, 0, 2, 3, 4).reshape(COUT, CIN, KK, KK)
    wl = np.zeros((128, 13, 128), np.float32)  # [k, j, m]
    j = 0
    for dy in (-2, 0):
        for dx in (-2, -1, 0, 1, 2):
            wl[0:64, j, :] = Wd[:, :, dy + 2, dx + 2].T
            wl[64:128, j, :] = Wd[:, :, dy + 3, dx + 2].T
            j += 1
    for dx0 in (-2, 0):
        wl[0:64, j, :] = Wd[:, :, 4, dx0 + 2].T
        wl[64:128, j, :] = Wd[:, :, 4, dx0 + 3].T
        j += 1
    wl[0:64, j, :] = Wd[:, :, 4, 4].T  # single tap (2,2) on lo partitions
    return wl


def _inputs_core(x, half):
    """x: [64, H, W] one image channel-major. Returns XA, XC [128, 68, 132]."""
    base = half * 64 - 2
    XA = np.zeros((128, XROWS, XCOLS), np.float32)
    XC = np.zeros((128, XROWS, XCOLS), np.float32)

    def fill(dst, roff, c0, c1):
        lo, hi = max(0, -(base + roff)), min(XROWS, H - base - roff)
        dst[:, lo:hi, c0:c1] = x[:, base + roff + lo:base + roff + hi, :]

    fill(XA[0:64], 0, 2, 130)
    fill(XA[64:128], 1, 2, 130)
    fill(XC[0:64], 2, 2, 130)
    fill(XC[64:128], 2, 1, 129)
    return XA, XC


def _s2_groups(half):
    """[32, len(GROUPS), BLK*128] f32: s^2 at partition m=8*bp+t1
    (t1-replicated), group g, flat pos = (row-within-block, col)."""
    idx = np.arange(H)
    cnt = (np.minimum(idx + 2, H - 1) - np.maximum(idx - 2, 0) + 1).astype(np.float64)
    s = 1.0 / (8.0 * cnt[:, None] * cnt[None, :])  # [H, W]
    s2 = (s * s)[half * 64:(half + 1) * 64, :]     # [64, 128]
    out = np.zeros((32, len(GROUPS), BLK * 128), np.float64)
    for g, (b0, ng) in enumerate(GROUPS):
        for bp in range(ng):
            blk = b0 + bp
            rows = s2[blk * BLK:(blk + 1) * BLK, :].reshape(-1)  # [512]
            out[8 * bp:8 * bp + 8, g, :] = rows[None, :]
    return np.ascontiguousarray(
        out.astype(np.float32).reshape(32, len(GROUPS) * BLK * 128))


def _bdv():
    """[128, N_BLKS*32]: c=(t1,z1) -> partition m=8*bp(blk)+t1, sum over z1."""
    bd = np.zeros((128, N_BLKS, 32), np.float32)
    c = np.arange(128)
    for blk in range(N_BLKS):
        bd[c, blk, 8 * _bp_of(blk) + c // 16] = 1.0
    return np.ascontiguousarray(bd.reshape(128, N_BLKS * 32))


def _exv():
    """[32, N_BLKS*128]: partition p=8*bp(blk)+t1 -> channels c, c//16==t1."""
    ex = np.zeros((32, N_BLKS, 128), np.float32)
    c = np.arange(128)
    for blk in range(N_BLKS):
        ex[8 * _bp_of(blk) + c // 16, blk, c] = 1.0
    return np.ascontiguousarray(ex.reshape(32, N_BLKS * 128))


def build_nc(reps=1):
    import concourse.bass as bass
    import concourse.bacc as bacc
    import concourse.mybir as mybir
    import concourse.tile as tile

    f32 = mybir.dt.float32
    f32r = mybir.dt.float32r
    AF = mybir.ActivationFunctionType
    NG = len(GROUPS)

    nc = bacc.Bacc(None, target_bir_lowering=False)
    xa_d = nc.dram_tensor("xa", [128, XROWS * XCOLS], f32r, kind="ExternalInput")
    xc_d = nc.dram_tensor("xc", [128, XROWS * XCOLS], f32r, kind="ExternalInput")
    wl_d = nc.dram_tensor("wl", [128, 13 * 128], f32r, kind="ExternalInput")
    bdv_d = nc.dram_tensor("bdv", [128, N_BLKS * 32], f32r, kind="ExternalInput")
    exv_d = nc.dram_tensor("exv", [32, N_BLKS * 128], f32r, kind="ExternalInput")
    s2_d = nc.dram_tensor("s2", [32, NG * BLK * 128], f32, kind="ExternalInput")
    out_d = nc.dram_tensor("out", [128, ROWS * 128], f32, kind="ExternalOutput")

    with tile.TileContext(nc) as tc:
        with (
            tc.tile_pool(name="consts", bufs=1) as consts,
            tc.tile_pool(name="sq", bufs=3) as sq,
            tc.tile_pool(name="psb", bufs=9) as psb,
            tc.tile_pool(name="fac", bufs=2) as fac,
            tc.tile_pool(name="ff", bufs=2) as ff,
            tc.tile_pool(name="vv", bufs=3) as vv,
            tc.tile_pool(name="pp", bufs=3, space="PSUM") as pp,
            tc.tile_pool(name="py", bufs=2, space="PSUM") as py,
            tc.tile_pool(name="pf", bufs=2, space="PSUM") as pf,
            tc.tile_pool(name="pd", bufs=1, space="PSUM") as pd,
        ):
            # PE pre-warm: dummy matmuls on a zeroed tile keep the PE busy
            # during the input-DMA lead-in so HAM un-throttles to 2.4 GHz
            # before the first real conv matmul.
            dum = consts.tile([128, 512], f32)
            nc.gpsimd.memset(dum[:], 0.0)
            dum_ps = pd.tile([128, 512], f32)
            for _ in range(14):
                nc.tensor.matmul(dum_ps[:], dum[:, 0:128].bitcast(f32r),
                                 dum[:].bitcast(f32r), start=True, stop=True)

            wl = consts.tile([128, 13, 128], f32r)
            wl_src = wl_d.ap().rearrange("p (j m) -> p j m", m=128)
            nc.sync.dma_start(out=wl, in_=wl_src)

            # Input chunks, all on the sync queue in strict priority order so
            # ring FIFO delivers critical bytes first. Tile 0 holds only
            # block 0's rows; tile i (1..8) covers blocks {2i-1, 2i}:
            # xa rows [8i-4, 8i+6), xc rows [8i-2, 8i+6).
            xa_src = xa_d.ap().rearrange("p (r c) -> p r c", c=XCOLS)
            xc_src = xc_d.ap().rearrange("p (r c) -> p r c", c=XCOLS)
            xat = [consts.tile([128, 6, XCOLS], f32r, name="xat0")] + [
                consts.tile([128, min(8 * i + 6, XROWS - 2) - (8 * i - 4),
                             XCOLS], f32r, name=f"xat{i}")
                for i in range(1, 9)]
            xct = [consts.tile([128, 4, XCOLS], f32r, name="xct0")] + [
                consts.tile([128, min(8 * i + 6, XROWS - 2) - (8 * i - 2),
                             XCOLS], f32r, name=f"xct{i}")
                for i in range(1, 9)]
            nc.sync.dma_start(out=xat[0][:], in_=xa_src[:, 0:6, :])
            nc.sync.dma_start(out=xct[0][:], in_=xc_src[:, 2:6, :])
            for i in range(1, 9):
                hi = min(8 * i + 6, XROWS - 2)
                nc.sync.dma_start(out=xat[i][:],
                                  in_=xa_src[:, 8 * i - 4:hi, :])
                nc.sync.dma_start(out=xct[i][:],
                                  in_=xc_src[:, 8 * i - 2:hi, :])

            bdv = consts.tile([128, N_BLKS, 32], f32r)
            nc.sync.dma_start(
                out=bdv, in_=bdv_d.ap().rearrange("p (b m) -> p b m", m=32))
            exv = consts.tile([32, N_BLKS, 128], f32r)
            nc.sync.dma_start(
                out=exv, in_=exv_d.ap().rearrange("p (b c) -> p b c", c=128))
            s2_sb = consts.tile([32, NG, BLK, 128], f32)
            nc.sync.dma_start(
                out=s2_sb,
                in_=s2_d.ap().rearrange("p (g r c) -> p g r c", r=BLK, c=128))
            eps_t = consts.tile([32, 1], f32)
            nc.gpsimd.memset(eps_t[:], 1e-9)

            out_v = out_d.ap().rearrange("p (r c) -> p r c", c=128)

            import contextlib
            loop_ctx = (tc.For_i(0, reps, 1,
                                 hint_engines=(mybir.EngineType.PE,
                                               mybir.EngineType.DVE,
                                               mybir.EngineType.Activation,
                                               mybir.EngineType.Pool,
                                               mybir.EngineType.SP))
                        if reps > 1 else contextlib.nullcontext())

            def conv_group(gi, b0, ng, psbs):
                y_ps = py.tile([8 * ng, BLK, 128], f32)
                for bp in range(ng):
                    blk = b0 + bp
                    ci = 0 if blk == 0 else (blk + 1) // 2
                    ro = 0 if (blk == 0 or blk % 2 == 1) else 4
                    p_ps = pp.tile([128, BLK, 128], f32)
                    for j, (src, roff, coff) in enumerate(_MM_SLICES):
                        if src == 'XA':
                            xt, r = xat[ci], ro + roff
                        else:
                            xt, r = xct[ci], ro + roff - 2
                        if j == 12:  # K=64 single on lo partitions
                            lhsT = wl[0:64, j, :]
                            rhs = xt[0:64, r:r + BLK, coff:coff + 128]
                        else:
                            lhsT = wl[:, j, :]
                            rhs = xt[:, r:r + BLK, coff:coff + 128]
                        nc.tensor.matmul(p_ps[:], lhsT, rhs,
                                         start=(j == 0), stop=(j == 12))
                    psq = sq.tile([128, BLK, 128], f32r, tag="psq")
                    nc.scalar.activation(psq[:], p_ps[:], AF.Square)
                    p_sb = psb.tile([128, BLK, 128], f32, tag="psb")
                    nc.scalar.activation(p_sb[:], p_ps[:], AF.Copy, bias=0.0)
                    psbs[blk] = p_sb
                    nc.tensor.matmul(y_ps[:], bdv[:, blk, 0:8 * ng], psq[:],
                                     start=(bp == 0), stop=(bp == ng - 1))
                return y_ps

            def factor(gi, ng, y_ps):
                # F = y/((1+y)*sqrt(y_raw+eps)), y = s^2*y_raw, on [8ng, 512]
                P = 8 * ng
                a_t = fac.tile([P, BLK, 128], f32, tag="a")
                nc.scalar.activation(a_t[:], y_ps[:], AF.Sqrt,
                                     bias=eps_t[0:P, :])
                y_t = fac.tile([P, BLK, 128], f32, tag="y")
                nc.vector.tensor_mul(y_t[:], y_ps[:], s2_sb[0:P, gi, :, :])
                y1_t = fac.tile([P, BLK, 128], f32, tag="y1")
                nc.scalar.activation(y1_t[:], y_t[:], AF.Copy, bias=1.0)
                b_t = fac.tile([P, BLK, 128], f32, tag="b")
                nc.vector.tensor_mul(b_t[:], a_t[:], y1_t[:])
                r_t = fac.tile([P, BLK, 128], f32, tag="r")
                nc.vector.reciprocal_approx_fast(r_t[:], b_t[:])
                F_t = ff.tile([P, BLK, 128], f32r, tag="F")
                nc.vector.tensor_mul(F_t[:], y_t[:], r_t[:])
                return F_t

            def expand(b0, ng, F_t, psbs):
                for bp in range(ng):
                    blk = b0 + bp
                    r0 = blk * BLK
                    fe_ps = pf.tile([128, BLK, 128], f32)
                    nc.tensor.matmul(fe_ps[:], exv[0:8 * ng, blk, :], F_t[:],
                                     start=True, stop=True)
                    v_t = vv.tile([128, BLK, 128], f32, tag="v")
                    nc.vector.tensor_mul(v_t[:], psbs.pop(blk)[:], fe_ps[:])
                    nc.sync.dma_start(out=out_v[:, r0:r0 + BLK, :], in_=v_t[:])

            with loop_ctx:
                psbs = {}
                prev = None
                for gi, (b0, ng) in enumerate(GROUPS):
                    y_ps = conv_group(gi, b0, ng, psbs)
                    if prev is not None:
                        expand(prev[0], prev[1], prev[2], psbs)
                    prev = (b0, ng, factor(gi, ng, y_ps))
                expand(prev[0], prev[1], prev[2], psbs)

    nc.compile()
    return nc


def _prep_in_maps(u, W):
    x = u.reshape(N, CIN, H, W_SP)
    wl = _weight_tiles(W).reshape(128, 13 * 128)
    bdv = _bdv()
    exv = _exv()
    s2q = [_s2_groups(half) for half in range(2)]
    in_maps = []
    for core in range(N_CORES):
        n, half = core // 2, core % 2
        XA, XC = _inputs_core(x[n], half)
        in_maps.append({
            "xa": XA.reshape(128, XROWS * XCOLS),
            "xc": XC.reshape(128, XROWS * XCOLS),
            "wl": wl,
            "bdv": bdv,
            "exv": exv,
            "s2": s2q[half],
        })
    return in_maps


def run(u, W, trace=False):
    """Returns (out [N,T1,Z1,H,W] f32, BassKernelResults)."""
    from concourse.bass_utils import run_bass_kernel_spmd

    if "nc" not in _CACHE:
        _CACHE["nc"] = build_nc()
    nc = _CACHE["nc"]
    in_maps = _prep_in_maps(np.asarray(u, np.float32), np.asarray(W, np.float32))
    res = run_bass_kernel_spmd(nc, in_maps, list(range(N_CORES)), trace=trace)
    out = np.empty((N, T1, Z1, H, W_SP), np.float32)
    for core in range(N_CORES):
        n, half = core // 2, core % 2
        o = res.results[core]["out"].reshape(T1, Z1, ROWS, 128)
        out[n, :, :, half * 64:(half + 1) * 64, :] = o
    return out, res


def kernel(u, W):
    out, _ = run(u, W, trace=False)
    return out
